# revision 8
# baseline (speedup 1.0000x reference)
"""Cross-attention kernel for Trainium2: B=8 data-parallel across 8 NeuronCores.

Per core (one example): q = h@Wq+bq, k = x@Wk+bk, v = x@Wv,
s = q@k^T (output), e = exp(s^T) computed transposed, rowsum via ones-matmul,
o2n = (e^T@v / rowsum) @ Wo (output).  Host: out = gamma*o2n + lam*x
+ gamma*(bias_v@Wo + bias_o);  returns (out, s, gamma).

Raw bass (no Tile): explicit engine programs + semaphores (this walrus
rejects multi-wait instructions, so every wait is a standalone wait_ge).
All matmuls fp32r (1 cyc/row @ N=512).

PSUM plan (8 banks as 3 tensors):
  psA [128,1024] (b0-1): sT pairs -> batched exp; qT pairs (phase A); rowsum in [0:1,0:512]
  psB [128,1024] (b2-3): kT pairs (phase A); oT sequential cc; o2 rotation
  psC [128,2048] (b4-7): v (phase A, batches of 4); s_nat quads -> one DVE copy each
"""
import numpy as np

import concourse.bass as bass
from concourse import mybir
from concourse.bass_utils import run_bass_kernel_spmd

DT = mybir.dt
N = 2048      # sequence length
C = 512       # channels
F = 64        # q/k feature dim
NB = 4        # n-blocks of 512
NS = 4        # sub-blocks of 128 per n-block
MC = 16       # m-chunks of 128
CC = 4        # c-chunks of 128

AF = mybir.ActivationFunctionType


class Waiter:
    """Per-engine wait emitter that skips waits already dominated."""

    def __init__(self, eng):
        self.eng = eng
        self.seen = {}

    def wait(self, sem, val):
        key = id(sem)
        if self.seen.get(key, -1) >= val:
            return
        self.eng.wait_ge(sem, val)
        self.seen[key] = val


def build_nc():
    nc = bass.Bass("TRN2", target_bir_lowering=False, debug=False)

    # ---- DRAM params (per-core) ----
    ht_d = nc.declare_dram_parameter("ht", [C, N], DT.float32r, isOutput=False)
    xt_d = nc.declare_dram_parameter("xt", [C, N], DT.float32r, isOutput=False)
    wq_d = nc.declare_dram_parameter("wq", [C, F], DT.float32r, isOutput=False)
    wk_d = nc.declare_dram_parameter("wk", [C, F], DT.float32r, isOutput=False)
    wv_d = nc.declare_dram_parameter("wv", [C, C], DT.float32r, isOutput=False)
    wo_d = nc.declare_dram_parameter("wo", [C, C], DT.float32r, isOutput=False)
    bq_d = nc.declare_dram_parameter("bq", [F, 1], DT.float32, isOutput=False)
    bk_d = nc.declare_dram_parameter("bk", [F, 1], DT.float32, isOutput=False)
    ones_d = nc.declare_dram_parameter("ones", [128, 1], DT.float32r, isOutput=False)
    s_d = nc.declare_dram_parameter("s", [N, N], DT.float32, isOutput=True)
    o2n_d = nc.declare_dram_parameter("o2n", [N, C], DT.float32, isOutput=True)

    # ---- SBUF ----
    xt_s = nc.alloc_sbuf_tensor("xt_s", [128, CC, N], DT.float32r)
    hta_s = nc.alloc_sbuf_tensor("hta_s", [128, CC, 512], DT.float32r)
    htb_s = nc.alloc_sbuf_tensor("htb_s", [128, CC, 512], DT.float32r)
    wq_s = nc.alloc_sbuf_tensor("wq_s", [128, CC, F], DT.float32r)
    wk_s = nc.alloc_sbuf_tensor("wk_s", [128, CC, F], DT.float32r)
    wv_s = nc.alloc_sbuf_tensor("wv_s", [128, CC, C], DT.float32r)
    wo_s = nc.alloc_sbuf_tensor("wo_s", [128, CC, C], DT.float32r)
    qt_s = nc.alloc_sbuf_tensor("qt_s", [F, N], DT.float32r)
    kt_s = nc.alloc_sbuf_tensor("kt_s", [F, N], DT.float32r)
    v_s = nc.alloc_sbuf_tensor("v_s", [128, MC, C], DT.float32r)
    et_s = nc.alloc_sbuf_tensor("et_s", [128, MC, 512], DT.float32r)
    ot_s = nc.alloc_sbuf_tensor("ot_s", [128, CC, 512], DT.float32r)
    ssta_s = nc.alloc_sbuf_tensor("ssta_s", [128, N], DT.float32)
    sstb_s = nc.alloc_sbuf_tensor("sstb_s", [128, N], DT.float32)
    outa_s = nc.alloc_sbuf_tensor("outa_s", [128, C], DT.float32)
    outb_s = nc.alloc_sbuf_tensor("outb_s", [128, C], DT.float32)
    bq_s = nc.alloc_sbuf_tensor("bq_s", [F, 1], DT.float32)
    bk_s = nc.alloc_sbuf_tensor("bk_s", [F, 1], DT.float32)
    ones_s = nc.alloc_sbuf_tensor("ones_s", [128, 1], DT.float32r)
    rrow_s = nc.alloc_sbuf_tensor("rrow_s", [1, 512], DT.float32)
    rcol_s = nc.alloc_sbuf_tensor("rcol_s", [128, NS], DT.float32)

    hstage = [hta_s, htb_s]
    sstage = [ssta_s, sstb_s]
    ostage = [outa_s, outb_s]

    # ---- PSUM ----
    psA = nc.alloc_psum_tensor("psA", [128, 1024], DT.float32)
    psB = nc.alloc_psum_tensor("psB", [128, 1024], DT.float32)
    psC = nc.alloc_psum_tensor("psC", [128, 2048], DT.float32)

    # ---- milestone bookkeeping ----
    # phase A PE: kt 1..4, qt0/1 5..6, v0..7 7..14, qt2/3 15..16, v8..15 17..24
    kt_pe = {n4: n4 + 1 for n4 in range(4)}
    qt_pe = {0: 5, 1: 6, 2: 15, 3: 16}
    v_pe = {}
    for mc in range(8):
        v_pe[mc] = 7 + mc
    for mc in range(8, 16):
        v_pe[mc] = 17 + (mc - 8)
    pe_c = 24
    # phase A ACT: kt-p0, kt-p1, qt-p0, v-b0, v-b1, qt-p1, v-b2, v-b3
    ktp_ev = {0: 1, 1: 2}
    qtp_ev = {0: 3, 1: 6}
    vb_ev = {0: 4, 1: 5, 2: 7, 3: 8}
    act_c = 8
    phaseA_act = 8

    st_pe = {}; sn_pe = {}; rs_pe = {}; ot_pe = {}; o2_pe = {}
    exp_ev = {}; otp_ev = {}
    snc_dv = {}; rcp_dv = {}; o2_dv = {}
    dve_c = 0
    for nb in range(NB):
        for g in range(12):
            if g % 3 == 2:
                pe_c += 1; sn_pe[(nb, g // 3)] = pe_c
            else:
                pe_c += 1; st_pe[(nb, (g // 3) * 2 + (g % 3))] = pe_c
        pe_c += 1; rs_pe[nb] = pe_c
        for cc in range(CC):
            pe_c += 1; ot_pe[(nb, cc)] = pe_c
        for ns in range(NS):
            pe_c += 1; o2_pe[(nb, ns)] = pe_c
        for p in range(8):
            act_c += 1; exp_ev[(nb, p)] = act_c
        for t in range(2):
            act_c += 1; otp_ev[(nb, t)] = act_c
        for q in range(4):
            dve_c += 1; snc_dv[(nb, q)] = dve_c
        dve_c += 1; rcp_dv[nb] = dve_c
        for ns in range(NS):
            dve_c += 1; o2_dv[(nb, ns)] = dve_c

    with (
        nc.Block() as block,
        nc.semaphore("s_sm") as s_sm,
        nc.semaphore("s_x0") as s_x0,
        nc.semaphore("s_x1") as s_x1,
        nc.semaphore("s_x2") as s_x2,
        nc.semaphore("s_x3") as s_x3,
        nc.semaphore("s_wv") as s_wv,
        nc.semaphore("s_wo") as s_wo,
        nc.semaphore("s_ha") as s_ha,
        nc.semaphore("s_hb") as s_hb,
        nc.semaphore("pe_s") as pe_s,
        nc.semaphore("act_s") as act_s,
        nc.semaphore("dve_s") as dve_s,
        nc.semaphore("s_soa") as s_soa,
        nc.semaphore("s_sob") as s_sob,
        nc.semaphore("s_ooa") as s_ooa,
        nc.semaphore("s_oob") as s_oob,
        nc.semaphore("s_rc") as s_rc,
    ):
        @block.sync
        def _(eng: bass.BassEngine):
            w = Waiter(eng)
            eng.dma_start(out=wk_s[:, :, :],
                          in_=wk_d.rearrange("(cc p) f -> p cc f", p=128)[:, :, :]).then_inc(s_sm, 16)
            eng.dma_start(out=wq_s[:, :, :],
                          in_=wq_d.rearrange("(cc p) f -> p cc f", p=128)[:, :, :]).then_inc(s_sm, 16)
            eng.dma_start(out=bq_s[:, :], in_=bq_d[:, :]).then_inc(s_sm, 16)
            eng.dma_start(out=bk_s[:, :], in_=bk_d[:, :]).then_inc(s_sm, 16)
            eng.dma_start(out=ones_s[:, :], in_=ones_d[:, :]).then_inc(s_sm, 16)
            xtv = xt_d.rearrange("(cc p) n -> p cc n", p=128)
            xsems = [s_x0, s_x1, s_x2, s_x3]
            for c4 in range(4):
                eng.dma_start(out=xt_s[:, :, c4 * 512:(c4 + 1) * 512],
                              in_=xtv[:, :, c4 * 512:(c4 + 1) * 512]).then_inc(xsems[c4], 16)
            eng.dma_start(out=wv_s[:, :, :],
                          in_=wv_d.rearrange("(cc p) c -> p cc c", p=128)[:, :, :]).then_inc(s_wv, 16)
            htv = ht_d.rearrange("(cc p) n -> p cc n", p=128)
            eng.dma_start(out=hta_s[:, :, :], in_=htv[:, :, 0:512]).then_inc(s_ha, 16)
            eng.dma_start(out=htb_s[:, :, :], in_=htv[:, :, 512:1024]).then_inc(s_hb, 16)
            eng.dma_start(out=wo_s[:, :, :],
                          in_=wo_d.rearrange("(cc p) c -> p cc c", p=128)[:, :, :]).then_inc(s_wo, 16)
            w.wait(pe_s, qt_pe[0])
            eng.dma_start(out=hta_s[:, :, :], in_=htv[:, :, 1024:1536]).then_inc(s_ha, 16)
            w.wait(pe_s, qt_pe[1])
            eng.dma_start(out=htb_s[:, :, :], in_=htv[:, :, 1536:2048]).then_inc(s_hb, 16)

            for nb in range(NB):
                for q in range(4):
                    g = nb * NS + q
                    w.wait(dve_s, snc_dv[(nb, q)])
                    eng.dma_start(out=s_d[g * 128:(g + 1) * 128, :],
                                  in_=sstage[g % 2][:, :]).then_inc(
                                      s_soa if g % 2 == 0 else s_sob, 16)

        @block.tensor
        def _(eng: bass.BassEngine):
            w = Waiter(eng)
            w.wait(s_sm, 80)
            xsems = [s_x0, s_x1, s_x2, s_x3]
            for n4 in range(4):
                sl = psB[0:F, (n4 % 2) * 512:(n4 % 2) * 512 + 512]
                w.wait(xsems[n4], 16)
                if n4 >= 2:
                    w.wait(act_s, ktp_ev[0])
                for cc in range(CC):
                    mm = eng.matmul(sl, wk_s[:, cc, :], xt_s[:, cc, n4 * 512:(n4 + 1) * 512],
                                    start=(cc == 0), stop=(cc == CC - 1))
                mm.then_inc(pe_s, 1)
            for n4 in (0, 1):
                sl = psA[0:F, (n4 % 2) * 512:(n4 % 2) * 512 + 512]
                w.wait(s_ha if n4 % 2 == 0 else s_hb, 16)
                for cc in range(CC):
                    mm = eng.matmul(sl, wq_s[:, cc, :], hstage[n4 % 2][:, cc, :],
                                    start=(cc == 0), stop=(cc == CC - 1))
                mm.then_inc(pe_s, 1)
            w.wait(s_wv, 16)
            for sx in xsems:
                w.wait(sx, 16)
            for mc in range(8):
                sl = psC[:, (mc % 4) * 512:(mc % 4) * 512 + 512]
                if mc >= 4:
                    w.wait(act_s, vb_ev[mc // 4 - 1])
                for cc in range(CC):
                    mm = eng.matmul(sl, xt_s[:, cc, mc * 128:(mc + 1) * 128], wv_s[:, cc, :],
                                    start=(cc == 0), stop=(cc == CC - 1))
                mm.then_inc(pe_s, 1)
            for n4 in (2, 3):
                sl = psA[0:F, (n4 % 2) * 512:(n4 % 2) * 512 + 512]
                w.wait(s_ha if n4 % 2 == 0 else s_hb, 32)
                w.wait(act_s, qtp_ev[0])
                for cc in range(CC):
                    mm = eng.matmul(sl, wq_s[:, cc, :], hstage[n4 % 2][:, cc, :],
                                    start=(cc == 0), stop=(cc == CC - 1))
                mm.then_inc(pe_s, 1)
            for mc in range(8, 16):
                sl = psC[:, (mc % 4) * 512:(mc % 4) * 512 + 512]
                w.wait(act_s, vb_ev[mc // 4 - 1])
                for cc in range(CC):
                    mm = eng.matmul(sl, xt_s[:, cc, mc * 128:(mc + 1) * 128], wv_s[:, cc, :],
                                    start=(cc == 0), stop=(cc == CC - 1))
                mm.then_inc(pe_s, 1)

            # ---- phase B ----
            w.wait(act_s, phaseA_act)
            for nb in range(NB):
                for g in range(12):
                    if g % 3 == 2:
                        q = g // 3
                        if nb == 0 and q == 0:
                            pass
                        elif q == 0:
                            w.wait(dve_s, snc_dv[(nb - 1, 3)])
                        else:
                            w.wait(dve_s, snc_dv[(nb, q - 1)])
                        row = (nb * NS + q) * 128
                        for m4 in range(4):
                            mm = eng.matmul(psC[:, m4 * 512:(m4 + 1) * 512],
                                            qt_s[:, row:row + 128],
                                            kt_s[:, m4 * 512:(m4 + 1) * 512],
                                            start=True, stop=True)
                        mm.then_inc(pe_s, 1)
                    else:
                        p = (g // 3) * 2 + (g % 3)
                        ps_st = psA if p % 2 == 0 else psB
                        if nb == 0 and p < 2:
                            pass  # phase A evictions covered by phaseA_act
                        elif p == 0:
                            w.wait(act_s, exp_ev[(nb - 1, 6)])
                            w.wait(dve_s, rcp_dv[nb - 1])
                        elif p == 1:
                            w.wait(act_s, exp_ev[(nb - 1, 7)])
                            w.wait(dve_s, o2_dv[(nb - 1, 3)])
                        else:
                            w.wait(act_s, exp_ev[(nb, p - 2)])
                        for j in range(2):
                            mc = 2 * p + j
                            mm = eng.matmul(ps_st[:, j * 512:(j + 1) * 512],
                                            kt_s[:, mc * 128:(mc + 1) * 128],
                                            qt_s[:, nb * 512:(nb + 1) * 512],
                                            start=True, stop=True)
                        mm.then_inc(pe_s, 1)
                w.wait(act_s, exp_ev[(nb, 7)])
                for mc in range(MC):
                    mm = eng.matmul(psA[0:1, 0:512], ones_s[:, :], et_s[:, mc, :],
                                    start=(mc == 0), stop=(mc == MC - 1))
                mm.then_inc(pe_s, 1)
                for cc in range(CC):
                    if nb == 0 and cc < 2:
                        pass
                    elif cc < 2:
                        w.wait(dve_s, o2_dv[(nb - 1, cc + 2)])
                    else:
                        w.wait(act_s, otp_ev[(nb, 0)])
                    for mc in range(MC):
                        mm = eng.matmul(psB[:, (cc % 2) * 512:(cc % 2) * 512 + 512],
                                        v_s[:, mc, cc * 128:(cc + 1) * 128],
                                        et_s[:, mc, :], start=(mc == 0), stop=(mc == MC - 1))
                    mm.then_inc(pe_s, 1)
                w.wait(s_wo, 16)
                for ns in range(NS):
                    if ns < 2:
                        w.wait(act_s, otp_ev[(nb, 1)])
                    else:
                        w.wait(dve_s, o2_dv[(nb, ns - 2)])
                    for cc in range(CC):
                        mm = eng.matmul(psB[:, (ns % 2) * 512:(ns % 2) * 512 + 512],
                                        ot_s[:, cc, ns * 128:(ns + 1) * 128],
                                        wo_s[:, cc, :], start=(cc == 0), stop=(cc == CC - 1))
                    mm.then_inc(pe_s, 1)

        @block.scalar
        def _(eng: bass.BassEngine):
            w = Waiter(eng)
            w.wait(s_sm, 80)
            for t in range(2):
                w.wait(pe_s, kt_pe[2 * t + 1])
                eng.activation(kt_s[:, t * 1024:(t + 1) * 1024], psB[0:F, :],
                               AF.Identity, bias=bk_s[:, 0:1]).then_inc(act_s)
            w.wait(pe_s, qt_pe[1])
            eng.activation(qt_s[:, 0:1024], psA[0:F, :],
                           AF.Identity, bias=bq_s[:, 0:1]).then_inc(act_s)
            for b in range(2):
                w.wait(pe_s, v_pe[4 * b + 3])
                eng.activation(v_s[:, 4 * b:4 * b + 4, :], psC[:, :], AF.Copy).then_inc(act_s)
            w.wait(pe_s, qt_pe[3])
            eng.activation(qt_s[:, 1024:2048], psA[0:F, :],
                           AF.Identity, bias=bq_s[:, 0:1]).then_inc(act_s)
            for b in range(2, 4):
                w.wait(pe_s, v_pe[4 * b + 3])
                eng.activation(v_s[:, 4 * b:4 * b + 4, :], psC[:, :], AF.Copy).then_inc(act_s)

            for nb in range(NB):
                for p in range(8):
                    w.wait(pe_s, st_pe[(nb, p)])
                    eng.activation(et_s[:, 2 * p:2 * p + 2, :],
                                   (psA if p % 2 == 0 else psB)[:, :], AF.Exp).then_inc(act_s)
                for t in range(2):
                    w.wait(pe_s, ot_pe[(nb, 2 * t + 1)])
                    eng.activation(ot_s[:, 2 * t:2 * t + 2, :], psB[:, :], AF.Copy).then_inc(act_s)
                w.wait(dve_s, rcp_dv[nb])
                for c in range(4):
                    eng.dma_start(out=rcol_s[:, c:c + 1],
                                  in_=rrow_s[0:1, 128 * c:128 * (c + 1)]).then_inc(s_rc, 16)
                for ns in range(NS):
                    g = nb * NS + ns
                    w.wait(dve_s, o2_dv[(nb, ns)])
                    eng.dma_start(out=o2n_d[g * 128:(g + 1) * 128, :],
                                  in_=ostage[g % 2][:, :]).then_inc(
                                      s_ooa if g % 2 == 0 else s_oob, 16)

        @block.vector
        def _(eng: bass.BassEngine):
            w = Waiter(eng)
            for nb in range(NB):
                for q in range(4):
                    g = nb * NS + q
                    w.wait(pe_s, sn_pe[(nb, q)])
                    if g >= 2:
                        w.wait(s_soa if g % 2 == 0 else s_sob, 16 * (g // 2))
                    eng.tensor_copy(sstage[g % 2][:, :], psC[:, :]).then_inc(dve_s)
                w.wait(pe_s, rs_pe[nb])
                if nb > 0:
                    w.wait(s_rc, 64 * nb)
                eng.reciprocal(rrow_s[0:1, :], psA[0:1, 0:512]).then_inc(dve_s)
                for ns in range(NS):
                    g = nb * NS + ns
                    w.wait(pe_s, o2_pe[(nb, ns)])
                    w.wait(s_rc, 64 * (nb + 1))
                    if g >= 2:
                        w.wait(s_ooa if g % 2 == 0 else s_oob, 16 * (g // 2))
                    eng.tensor_scalar(out=ostage[g % 2][:, :],
                                      in0=psB[:, (ns % 2) * 512:(ns % 2) * 512 + 512],
                                      scalar1=rcol_s[:, ns:ns + 1], scalar2=None,
                                      op0=mybir.AluOpType.mult).then_inc(dve_s)

    return nc


_NC_CACHE = None


def _get_nc():
    global _NC_CACHE
    if _NC_CACHE is None:
        _NC_CACHE = build_nc()
    return _NC_CACHE


def _make_in_maps(inputs):
    h = np.asarray(inputs["h"], np.float32)
    x = np.asarray(inputs["x"], np.float32)
    wq = np.ascontiguousarray(np.asarray(inputs["kernel_q"], np.float32)[0])
    wk = np.ascontiguousarray(np.asarray(inputs["kernel_k"], np.float32)[0])
    wv = np.ascontiguousarray(np.asarray(inputs["kernel_v"], np.float32)[0])
    wo = np.ascontiguousarray(np.asarray(inputs["kernel_o"], np.float32)[0])
    bq = np.asarray(inputs["bias_q"], np.float32).reshape(F, 1)
    bk = np.asarray(inputs["bias_k"], np.float32).reshape(F, 1)
    ones = np.ones((128, 1), np.float32)
    in_maps = []
    for b in range(h.shape[0]):
        in_maps.append({
            "ht": np.ascontiguousarray(h[b].T),
            "xt": np.ascontiguousarray(x[b].T),
            "wq": wq, "wk": wk, "wv": wv, "wo": wo,
            "bq": bq, "bk": bk, "ones": ones,
        })
    return in_maps


def kernel(h, x, kernel_q, kernel_k, kernel_v, kernel_o,
           bias_q, bias_k, bias_v, bias_o, gamma, lam):
    h = np.asarray(h, dtype=np.float32)
    x = np.asarray(x, dtype=np.float32)
    wo = np.ascontiguousarray(np.asarray(kernel_o, np.float32)[0])
    bv = np.asarray(bias_v, np.float32).reshape(1, C)
    bo = np.asarray(bias_o, np.float32).reshape(1, C)
    gamma = np.asarray(gamma, np.float32)
    lam = np.asarray(lam, np.float32)

    nc = _get_nc()
    B = h.shape[0]
    in_maps = _make_in_maps({"h": h, "x": x, "kernel_q": kernel_q,
                             "kernel_k": kernel_k, "kernel_v": kernel_v,
                             "kernel_o": kernel_o, "bias_q": bias_q,
                             "bias_k": bias_k})
    res = run_bass_kernel_spmd(nc, in_maps, list(range(B)))

    s = np.stack([res.results[b]["s"] for b in range(B)])
    o2n = np.stack([res.results[b]["o2n"] for b in range(B)])

    const_row = (bv @ wo + bo)
    out = gamma.reshape(1, 1, 1) * (o2n + const_row[None]) + lam.reshape(1, 1, 1) * x
    return (out.astype(np.float32), s.astype(np.float32), gamma)
